# revision 1
# baseline (speedup 1.0000x reference)
"""Trainium2 Bass kernel for nn_BatchProgramClassifier.

Reference computation (B=64, L=64, NPT=127, D=128, VOCAB=30000, LABELS=30):
  1. e = emb[tokens] @ Wc + bc                     per tree node
  2. h = bottom-up subtree sums of e (heap tree)   [B, L, NPT, D]
  3. enc = relu(max over nodes of h)               [B, L, D]
  4. masked single-head self-attention over L      [B, L, D]
  5. logits = (max over L) @ Wl + bl               [B, LABELS]

Sharding: data-parallel over batch, 8 batches per core across 8 cores.

Per-core device program (fp16 matmul operands, f32 PSUM accumulation):
  - dma_gather in transpose mode pulls fp16 embedding rows from HBM straight
    into D-major layout: e^T [128=D, tokens]. Each tree occupies 128 columns
    (127 nodes + 1 pad column that is never read).
  - One Wc-stationary matmul per 512 columns (4 per chunk): e'^T = Wc^T e^T
    into PSUM; the PSUM->SBUF copy on ACT folds the +bc bias (per-partition
    activation bias), writing fp16 into a 128-tree block buffer.
  - The heap-tree subtree sums run in place on DVE with level-by-level
    strided adds across all 128 trees of a block at once; one reduce_max per
    block over the node axis gives enc^T columns; ReLU once at the end.
  - Attention is batched: q/k/v and the final Wo matmul each run over all
    8 batches in one instruction; softmax runs on [64, 512] tiles with
    broadcast max/recip; only scores/attn-transpose/attn@v are per batch.
"""

import math

import numpy as np

B, L, NPT, D_TREE = 64, 64, 127, 7
VOCAB, D, LABELS = 30000, 128, 30
NCORES = 8
BC = B // NCORES  # batches per core
TREES = BC * L  # trees per core
CHUNK_TREES = 16  # trees per gather chunk
NCHUNKS = TREES // CHUNK_TREES
NIDX_CHUNK = CHUNK_TREES * 128
NIDX_TOTAL = TREES * 128
MB_TREES = 128  # trees per tree-sum megablock
NMB = TREES // MB_TREES

_CACHE = {}


def _build_nc():
    import concourse.bacc as bacc
    import concourse.mybir as mybir
    import concourse.tile as tile
    from concourse.library_config import mlp

    f32 = mybir.dt.float32
    f16 = mybir.dt.float16
    nc = bacc.Bacc(
        "TRN2",
        target_bir_lowering=False,
        debug=False,
        num_devices=NCORES,
        num_swdge_queues=2,
    )

    emb_d = nc.dram_tensor("emb", [VOCAB, D], f16, kind="ExternalInput")
    idx_d = nc.dram_tensor(
        "idxs", [128, NIDX_TOTAL // 16], mybir.dt.int16, kind="ExternalInput"
    )
    mask_d = nc.dram_tensor("mask", [L, BC * L], mybir.dt.int32, kind="ExternalInput")
    wc_d = nc.dram_tensor("wc", [D, D], f32, kind="ExternalInput")
    bcv_d = nc.dram_tensor("bcv", [D, 1], f32, kind="ExternalInput")
    wq_d = nc.dram_tensor("wq", [D, D], f32, kind="ExternalInput")
    wk_d = nc.dram_tensor("wk", [D, D], f32, kind="ExternalInput")
    wv_d = nc.dram_tensor("wv", [D, D], f32, kind="ExternalInput")
    wo_d = nc.dram_tensor("wo", [D, D], f32, kind="ExternalInput")
    wl_d = nc.dram_tensor("wl", [D, LABELS], f32, kind="ExternalInput")
    blb_d = nc.dram_tensor("blb", [BC, LABELS], f32, kind="ExternalInput")
    ident_d = nc.dram_tensor("ident", [128, 128], f32, kind="ExternalInput")
    out_d = nc.dram_tensor("out", [BC, LABELS], f32, kind="ExternalOutput")

    inv_sqrt_d = 1.0 / math.sqrt(float(D))

    with tile.TileContext(nc) as tc:
        with (
            tc.tile_pool(name="const", bufs=1) as cpool,
            tc.tile_pool(name="epool", bufs=4) as epool,
            tc.tile_pool(name="eblk", bufs=2) as eblkpool,
            tc.tile_pool(name="tpsum", bufs=2, space="PSUM") as tpsum,
            tc.tile_pool(name="apool", bufs=2) as apool,
            tc.tile_pool(name="apsum", bufs=1, space="PSUM") as apsum,
        ):
            nc.gpsimd.load_library(mlp)

            idx_t = cpool.tile([128, NIDX_TOTAL // 16], mybir.dt.int16, tag="idxs")

            def load_const(dram, shape, dtype):
                t = cpool.tile(shape, dtype, tag=dram.name)
                if dtype == dram.dtype:
                    nc.sync.dma_start(out=t[:], in_=dram[:])
                else:
                    raw = cpool.tile(shape, dram.dtype, tag=dram.name + "_raw")
                    nc.sync.dma_start(out=raw[:], in_=dram[:])
                    nc.scalar.copy(out=t[:], in_=raw[:])
                return t

            wc_t = load_const(wc_d, [D, D], f16)
            bcv_t = load_const(bcv_d, [D, 1], f32)
            wq_t = load_const(wq_d, [D, D], f16)
            wk_t = load_const(wk_d, [D, D], f16)
            wv_t = load_const(wv_d, [D, D], f16)
            wo_t = load_const(wo_d, [D, D], f16)
            wl_t = load_const(wl_d, [D, LABELS], f16)
            blb_t = load_const(blb_d, [BC, LABELS], f32)
            ident_t = load_const(ident_d, [128, 128], f32)
            mask_t = load_const(mask_d, [L, BC * L], mybir.dt.int32)

            # additive mask: 0 where mask>0, -1e9 where mask==0
            maskf = cpool.tile([L, BC * L], f32, tag="maskf")
            nc.vector.tensor_copy(out=maskf[:], in_=mask_t[:])
            nmask = cpool.tile([L, BC * L], f32, tag="nmask")
            nc.vector.tensor_scalar(
                nmask[:], maskf[:], 1e9, -1e9, mybir.AluOpType.mult, mybir.AluOpType.add
            )

            enc = cpool.tile([D, TREES], f16, tag="enc")  # enc^T, col = b*64+l

            # ---- tree phase ----
            idx_cols = NIDX_CHUNK // 16
            chunks_per_mb = MB_TREES // CHUNK_TREES
            for mb in range(NMB):
                eb = eblkpool.tile([128, MB_TREES * 128], f16, tag="eb")
                for k in range(chunks_per_mb):
                    c = mb * chunks_per_mb + k
                    nc.sync.dma_start(
                        out=idx_t[:, c * idx_cols : (c + 1) * idx_cols],
                        in_=idx_d[:, c * idx_cols : (c + 1) * idx_cols],
                    )
                    et = epool.tile([128, 1, NIDX_CHUNK], f16, tag="et")
                    nc.gpsimd.dma_gather(
                        et[:],
                        emb_d[:],
                        idx_t[:, c * idx_cols : (c + 1) * idx_cols],
                        NIDX_CHUNK,
                        NIDX_CHUNK,
                        D,
                        transpose=True,
                        single_packet=False,
                        queue_num=c % 2,
                    )
                    for j in range(NIDX_CHUNK // 512):
                        pp = tpsum.tile([128, 512], f32, tag="pp")
                        nc.tensor.matmul(
                            pp[:],
                            lhsT=wc_t[:],
                            rhs=et[:, 0, j * 512 : (j + 1) * 512],
                            start=True,
                            stop=True,
                        )
                        off = k * NIDX_CHUNK + j * 512
                        # PSUM->SBUF copy with the +bc bias folded in
                        nc.scalar.activation(
                            eb[:, off : off + 512],
                            pp[:],
                            mybir.ActivationFunctionType.Identity,
                            bias=bcv_t[:],
                            scale=1.0,
                        )
                # in-place bottom-up subtree sums across all MB_TREES trees
                ebv = eb.rearrange("p (t n) -> p t n", n=128)
                for lvl in range(D_TREE - 2, -1, -1):
                    start, cnt = 2**lvl - 1, 2**lvl
                    tmp = epool.tile([128, MB_TREES, 32], f16, tag="tmp")
                    # children of nodes [start, start+cnt) are the 2*cnt
                    # consecutive nodes from 2*start+1, viewed as (node, pair)
                    chv = ebv[:, :, 2 * start + 1 : 2 * start + 1 + 2 * cnt].rearrange(
                        "p t (n two) -> p t n two", two=2
                    )
                    nc.vector.tensor_add(
                        out=tmp[:, :, :cnt],
                        in0=chv[:, :, :, 0],
                        in1=chv[:, :, :, 1],
                    )
                    nc.vector.tensor_add(
                        out=ebv[:, :, start : start + cnt],
                        in0=ebv[:, :, start : start + cnt],
                        in1=tmp[:, :, :cnt],
                    )
                nc.vector.reduce_max(
                    out=enc[:, mb * MB_TREES : (mb + 1) * MB_TREES],
                    in_=ebv[:, :, :NPT],
                    axis=mybir.AxisListType.X,
                )
            # ReLU (max with the zero background)
            nc.vector.tensor_scalar_max(enc[:], enc[:], 0.0)

            # ---- batched attention ----
            def wide_qkv(w_t, scale=None):
                ps = apsum.tile([D, BC * L], f32, tag="qkp")
                nc.tensor.matmul(ps[:], lhsT=w_t[:], rhs=enc[:], start=True, stop=True)
                if scale is None:
                    s = apool.tile([D, BC * L], f16, tag="qks")
                    nc.scalar.copy(out=s[:], in_=ps[:])
                else:
                    s = apool.tile([D, BC * L], f16, tag="qks")
                    nc.scalar.mul(s[:], ps[:], scale)
                return s

            qs = wide_qkv(wq_t, scale=inv_sqrt_d)
            ks = wide_qkv(wk_t)
            vp = apsum.tile([D, BC * L], f32, tag="qkp")
            nc.tensor.matmul(vp[:], lhsT=wv_t[:], rhs=enc[:], start=True, stop=True)
            vs = apool.tile([D, BC * L], f32, tag="vs")
            nc.scalar.copy(out=vs[:], in_=vp[:])

            # scores for all batches into one PSUM bank [64, 512]
            scp = apsum.tile([L, BC * L], f32, tag="scp")
            for b in range(BC):
                nc.tensor.matmul(
                    scp[:, b * L : (b + 1) * L],
                    lhsT=qs[:, b * L : (b + 1) * L],
                    rhs=ks[:, b * L : (b + 1) * L],
                    start=True,
                    stop=True,
                )
            sm = apool.tile([L, BC * L], f32, tag="sm")
            nc.vector.tensor_add(out=sm[:], in0=scp[:], in1=nmask[:])
            smv = sm.rearrange("q (b k) -> q b k", k=L)
            nrmax = apool.tile([L, BC], f32, tag="nrmax")
            nc.vector.reduce_max(
                out=nrmax[:], in_=smv, axis=mybir.AxisListType.X, negate=True
            )
            exn = apool.tile([L, BC * L], f32, tag="exn")
            nc.vector.tensor_add(
                out=exn.rearrange("q (b k) -> q b k", k=L),
                in0=smv,
                in1=nrmax[:, :, None].to_broadcast((L, BC, L)),
            )
            ex = apool.tile([L, BC * L], f32, tag="ex")
            nc.scalar.activation(ex[:], exn[:], mybir.ActivationFunctionType.Exp)
            rsum = apool.tile([L, BC], f32, tag="rsum")
            nc.vector.reduce_sum(
                out=rsum[:], in_=ex.rearrange("q (b k) -> q b k", k=L),
                axis=mybir.AxisListType.X,
            )
            rinv = apool.tile([L, BC], f32, tag="rinv")
            nc.vector.reciprocal(rinv[:], rsum[:])
            attn = apool.tile([L, BC * L], f32, tag="attn")
            nc.vector.tensor_mul(
                out=attn.rearrange("q (b k) -> q b k", k=L),
                in0=ex.rearrange("q (b k) -> q b k", k=L),
                in1=rinv[:, :, None].to_broadcast((L, BC, L)),
            )

            # per-batch transposes and attn@v, all into wide tiles
            op_all = apsum.tile([D, BC * L], f32, tag="opall")
            for b in range(BC):
                atp = apsum.tile([L, L], f32, tag="smallp", bufs=2)
                nc.tensor.transpose(atp[:], attn[:, b * L : (b + 1) * L], ident_t[:L, :L])
                ats = apool.tile([L, L], f16, tag="ats")
                nc.scalar.copy(out=ats[:], in_=atp[:])
                vtp = apsum.tile([L, D], f32, tag="smallp", bufs=2)
                nc.tensor.transpose(vtp[:], vs[:, b * L : (b + 1) * L], ident_t[:])
                vts = apool.tile([L, D], f16, tag="vts")
                nc.scalar.copy(out=vts[:], in_=vtp[:])
                nc.tensor.matmul(
                    op_all[:, b * L : (b + 1) * L],
                    lhsT=vts[:],
                    rhs=ats[:],
                    start=True,
                    stop=True,
                )
            os_all = apool.tile([D, BC * L], f16, tag="osall")
            nc.scalar.copy(out=os_all[:], in_=op_all[:])
            o2p = apsum.tile([D, BC * L], f32, tag="o2p")
            nc.tensor.matmul(o2p[:], lhsT=wo_t[:], rhs=os_all[:], start=True, stop=True)
            pooled = apool.tile([D, BC], f16, tag="pooled")
            nc.vector.reduce_max(
                out=pooled[:], in_=o2p.rearrange("d (b l) -> d b l", l=L),
                axis=mybir.AxisListType.X,
            )

            # ---- logits ----
            lgp = apsum.tile([BC, LABELS], f32, tag="smallp", bufs=2)
            nc.tensor.matmul(lgp[:], lhsT=pooled[:], rhs=wl_t[:], start=True, stop=True)
            outs = apool.tile([BC, LABELS], f32, tag="outs")
            nc.vector.tensor_add(out=outs[:], in0=lgp[:], in1=blb_t[:])
            nc.sync.dma_start(out=out_d[:], in_=outs[:])

    nc.compile()
    return nc


def _get_nc():
    if "nc" not in _CACHE:
        _CACHE["nc"] = _build_nc()
    return _CACHE["nc"]


def kernel(tokens, mask, emb, Wc, bc, Wq, Wk, Wv, Wo, Wl, bl, _trace=False):
    from concourse.bass_utils import run_bass_kernel_spmd

    tokens = np.asarray(tokens)
    mask = np.asarray(mask)
    emb16 = np.asarray(emb, dtype=np.float32).astype(np.float16)

    blb = np.tile(np.asarray(bl, np.float32)[None, :], (BC, 1))

    common = {
        "emb": emb16,
        "wc": np.asarray(Wc, np.float32),
        "bcv": np.asarray(bc, np.float32).reshape(D, 1),
        "wq": np.asarray(Wq, np.float32),
        "wk": np.asarray(Wk, np.float32),
        "wv": np.asarray(Wv, np.float32),
        "wo": np.asarray(Wo, np.float32),
        "wl": np.asarray(Wl, np.float32),
        "blb": blb,
        "ident": np.eye(128, dtype=np.float32),
    }

    in_maps = []
    for c in range(NCORES):
        tok_c = np.asarray(tokens[c * BC : (c + 1) * BC]).reshape(TREES, NPT)
        idx_lin = np.concatenate(
            [tok_c, np.zeros((TREES, 1), tok_c.dtype)], axis=1
        ).reshape(-1)
        idx_arr = np.tile(
            idx_lin.astype(np.int16).reshape(-1, 16).T, (8, 1)
        )  # [128, NIDX_TOTAL/16]
        mask_c = (
            np.asarray(mask[c * BC : (c + 1) * BC], np.int32)
            .transpose(1, 0, 2)
            .reshape(L, BC * L)
        )
        in_maps.append({**common, "idxs": idx_arr, "mask": mask_c})

    nc = _get_nc()
    res = run_bass_kernel_spmd(
        nc, in_maps, core_ids=list(range(NCORES)), trace=_trace
    )
    out = np.concatenate([r["out"] for r in res.results], axis=0)  # [B, LABELS]
    if _trace:
        return out, res
    return out



# revision 4
# speedup vs baseline: 1.7031x; 1.7031x over previous
"""Trainium2 Bass kernel for nn_BatchProgramClassifier.

Reference computation (B=64, L=64, NPT=127, D=128, VOCAB=30000, LABELS=30):
  1. e = emb[tokens] @ Wc + bc                     per tree node
  2. h = bottom-up subtree sums of e (heap tree)   [B, L, NPT, D]
  3. enc = relu(max over nodes of h)               [B, L, D]
  4. masked single-head self-attention over L      [B, L, D]
  5. logits = (max over L) @ Wl + bl               [B, LABELS]

Sharding: data-parallel over batch, 8 batches per core across 8 cores.

Per-core device program (fp16 matmul operands, f32 PSUM accumulation):
  - dma_gather in transpose mode pulls fp16 embedding rows from HBM straight
    into D-major layout: e^T [128=D, tokens]. Each tree occupies 128 columns
    (127 nodes + 1 pad column that is never read).
  - One Wc-stationary matmul per 512 columns (4 per chunk): e'^T = Wc^T e^T
    into PSUM; the PSUM->SBUF copy on ACT folds the +bc bias (per-partition
    activation bias), writing fp16 into a 128-tree block buffer.
  - The heap-tree subtree sums run in place on DVE with level-by-level
    strided adds across all 128 trees of a block at once; one reduce_max per
    block over the node axis gives enc^T columns; ReLU once at the end.
  - Attention is batched: q/k/v and the final Wo matmul each run over all
    8 batches in one instruction; softmax runs on [64, 512] tiles with
    broadcast max/recip; only scores/attn-transpose/attn@v are per batch.
"""

import math

import numpy as np

B, L, NPT, D_TREE = 64, 64, 127, 7
VOCAB, D, LABELS = 30000, 128, 30
NCORES = 8
BC = B // NCORES  # batches per core
TREES = BC * L  # trees per core
CHUNK_TREES = 16  # trees per gather chunk
NCHUNKS = TREES // CHUNK_TREES
NIDX_CHUNK = CHUNK_TREES * 128
NIDX_TOTAL = TREES * 128
MB_TREES = 128  # trees per tree-sum megablock
NMB = TREES // MB_TREES

_CACHE = {}


def _build_nc():
    import concourse.bacc as bacc
    import concourse.mybir as mybir
    import concourse.tile as tile
    from concourse.library_config import mlp

    f32 = mybir.dt.float32
    f16 = mybir.dt.float16
    nc = bacc.Bacc(
        "TRN2",
        target_bir_lowering=False,
        debug=False,
        num_devices=NCORES,
        num_swdge_queues=4,
    )

    emb_d = nc.dram_tensor("emb", [VOCAB, D], f16, kind="ExternalInput")
    idx_d = nc.dram_tensor(
        "idxs", [128, NIDX_TOTAL // 16], mybir.dt.int16, kind="ExternalInput"
    )
    mask_d = nc.dram_tensor("mask", [L, BC * L], mybir.dt.int32, kind="ExternalInput")
    wc_d = nc.dram_tensor("wc", [D, D], f32, kind="ExternalInput")
    bcv_d = nc.dram_tensor("bcv", [D, 1], f32, kind="ExternalInput")
    wq_d = nc.dram_tensor("wq", [D, D], f32, kind="ExternalInput")
    wk_d = nc.dram_tensor("wk", [D, D], f32, kind="ExternalInput")
    wv_d = nc.dram_tensor("wv", [D, D], f32, kind="ExternalInput")
    wo_d = nc.dram_tensor("wo", [D, D], f32, kind="ExternalInput")
    wl_d = nc.dram_tensor("wl", [D, LABELS], f32, kind="ExternalInput")
    blb_d = nc.dram_tensor("blb", [BC, LABELS], f32, kind="ExternalInput")
    ident_d = nc.dram_tensor("ident", [128, 128], f32, kind="ExternalInput")
    out_d = nc.dram_tensor("out", [BC, LABELS], f32, kind="ExternalOutput")

    inv_sqrt_d = 1.0 / math.sqrt(float(D))

    with tile.TileContext(nc) as tc:
        with (
            tc.tile_pool(name="const", bufs=1) as cpool,
            tc.tile_pool(name="epool", bufs=6) as epool,
            tc.tile_pool(name="eblk", bufs=2) as eblkpool,
            tc.tile_pool(name="tpsum", bufs=2, space="PSUM") as tpsum,
            tc.tile_pool(name="apool", bufs=2) as apool,
            tc.tile_pool(name="apsum", bufs=1, space="PSUM") as apsum,
        ):
            nc.gpsimd.load_library(mlp)

            idx_t = cpool.tile([128, NIDX_TOTAL // 16], mybir.dt.int16, tag="idxs")

            def load_const(dram, shape, dtype):
                t = cpool.tile(shape, dtype, tag=dram.name)
                if dtype == dram.dtype:
                    nc.sync.dma_start(out=t[:], in_=dram[:])
                else:
                    raw = cpool.tile(shape, dram.dtype, tag=dram.name + "_raw")
                    nc.sync.dma_start(out=raw[:], in_=dram[:])
                    nc.scalar.copy(out=t[:], in_=raw[:])
                return t

            wc_t = load_const(wc_d, [D, D], f16)
            bcv_t = load_const(bcv_d, [D, 1], f32)
            wq_t = load_const(wq_d, [D, D], f16)
            wk_t = load_const(wk_d, [D, D], f16)
            wv_t = load_const(wv_d, [D, D], f16)
            wo_t = load_const(wo_d, [D, D], f16)
            wl_t = load_const(wl_d, [D, LABELS], f16)
            blb_t = load_const(blb_d, [BC, LABELS], f32)
            ident_t = load_const(ident_d, [128, 128], f32)
            mask_t = load_const(mask_d, [L, BC * L], mybir.dt.int32)

            # additive mask: 0 where mask>0, -1e9 where mask==0
            maskf = cpool.tile([L, BC * L], f32, tag="maskf")
            nc.vector.tensor_copy(out=maskf[:], in_=mask_t[:])
            nmask = cpool.tile([L, BC * L], f32, tag="nmask")
            nc.vector.tensor_scalar(
                nmask[:], maskf[:], 1e9, -1e9, mybir.AluOpType.mult, mybir.AluOpType.add
            )

            enc = cpool.tile([D, TREES], f16, tag="enc")  # enc^T, col = b*64+l

            # ---- tree phase ----
            idx_cols = NIDX_CHUNK // 16
            chunks_per_mb = MB_TREES // CHUNK_TREES
            for mb in range(NMB):
                eb = eblkpool.tile([128, MB_TREES * 128], f16, tag="eb")
                for k in range(chunks_per_mb):
                    c = mb * chunks_per_mb + k
                    nc.sync.dma_start(
                        out=idx_t[:, c * idx_cols : (c + 1) * idx_cols],
                        in_=idx_d[:, c * idx_cols : (c + 1) * idx_cols],
                    )
                    et = epool.tile([128, 1, NIDX_CHUNK], f16, tag="et")
                    nc.gpsimd.dma_gather(
                        et[:],
                        emb_d[:],
                        idx_t[:, c * idx_cols : (c + 1) * idx_cols],
                        NIDX_CHUNK,
                        NIDX_CHUNK,
                        D,
                        transpose=True,
                        single_packet=False,
                        queue_num=c % 4,
                    )
                    for j in range(NIDX_CHUNK // 512):
                        pp = tpsum.tile([128, 512], f32, tag="pp")
                        nc.tensor.matmul(
                            pp[:],
                            lhsT=wc_t[:],
                            rhs=et[:, 0, j * 512 : (j + 1) * 512],
                            start=True,
                            stop=True,
                        )
                        off = k * NIDX_CHUNK + j * 512
                        # PSUM->SBUF copy with the +bc bias folded in
                        nc.scalar.activation(
                            eb[:, off : off + 512],
                            pp[:],
                            mybir.ActivationFunctionType.Identity,
                            bias=bcv_t[:],
                            scale=1.0,
                        )
                # in-place bottom-up subtree sums across all MB_TREES trees
                ebv = eb.rearrange("p (t n) -> p t n", n=128)
                for lvl in range(D_TREE - 2, -1, -1):
                    start, cnt = 2**lvl - 1, 2**lvl
                    tmp = epool.tile([128, MB_TREES, 32], f16, tag="tmp")
                    # children of nodes [start, start+cnt) are the 2*cnt
                    # consecutive nodes from 2*start+1, viewed as (node, pair)
                    chv = ebv[:, :, 2 * start + 1 : 2 * start + 1 + 2 * cnt].rearrange(
                        "p t (n two) -> p t n two", two=2
                    )
                    nc.vector.tensor_add(
                        out=tmp[:, :, :cnt],
                        in0=chv[:, :, :, 0],
                        in1=chv[:, :, :, 1],
                    )
                    nc.vector.tensor_add(
                        out=ebv[:, :, start : start + cnt],
                        in0=ebv[:, :, start : start + cnt],
                        in1=tmp[:, :, :cnt],
                    )
                nc.vector.reduce_max(
                    out=enc[:, mb * MB_TREES : (mb + 1) * MB_TREES],
                    in_=ebv[:, :, :NPT],
                    axis=mybir.AxisListType.X,
                )
            # ReLU (max with the zero background)
            nc.vector.tensor_scalar_max(enc[:], enc[:], 0.0)

            # ---- batched attention ----
            def wide_qkv(w_t, scale=None):
                ps = apsum.tile([D, BC * L], f32, tag="qkp")
                nc.tensor.matmul(ps[:], lhsT=w_t[:], rhs=enc[:], start=True, stop=True)
                if scale is None:
                    s = apool.tile([D, BC * L], f16, tag="qks")
                    nc.scalar.copy(out=s[:], in_=ps[:])
                else:
                    s = apool.tile([D, BC * L], f16, tag="qks")
                    nc.scalar.mul(s[:], ps[:], scale)
                return s

            qs = wide_qkv(wq_t, scale=inv_sqrt_d)
            ks = wide_qkv(wk_t)
            vp = apsum.tile([D, BC * L], f32, tag="qkp")
            nc.tensor.matmul(vp[:], lhsT=wv_t[:], rhs=enc[:], start=True, stop=True)
            vs = apool.tile([D, BC * L], f32, tag="vs")
            nc.scalar.copy(out=vs[:], in_=vp[:])

            # scores for all batches into one PSUM bank [64, 512]
            scp = apsum.tile([L, BC * L], f32, tag="scp")
            for b in range(BC):
                nc.tensor.matmul(
                    scp[:, b * L : (b + 1) * L],
                    lhsT=qs[:, b * L : (b + 1) * L],
                    rhs=ks[:, b * L : (b + 1) * L],
                    start=True,
                    stop=True,
                )
            sm = apool.tile([L, BC * L], f32, tag="sm")
            nc.vector.tensor_add(out=sm[:], in0=scp[:], in1=nmask[:])
            smv = sm.rearrange("q (b k) -> q b k", k=L)
            nrmax = apool.tile([L, BC], f32, tag="nrmax")
            nc.vector.reduce_max(
                out=nrmax[:], in_=smv, axis=mybir.AxisListType.X, negate=True
            )
            exn = apool.tile([L, BC * L], f32, tag="exn")
            nc.vector.tensor_add(
                out=exn.rearrange("q (b k) -> q b k", k=L),
                in0=smv,
                in1=nrmax[:, :, None].to_broadcast((L, BC, L)),
            )
            ex = apool.tile([L, BC * L], f32, tag="ex")
            nc.scalar.activation(ex[:], exn[:], mybir.ActivationFunctionType.Exp)
            rsum = apool.tile([L, BC], f32, tag="rsum")
            nc.vector.reduce_sum(
                out=rsum[:], in_=ex.rearrange("q (b k) -> q b k", k=L),
                axis=mybir.AxisListType.X,
            )
            rinv = apool.tile([L, BC], f32, tag="rinv")
            nc.vector.reciprocal(rinv[:], rsum[:])
            attn = apool.tile([L, BC * L], f32, tag="attn")
            nc.vector.tensor_mul(
                out=attn.rearrange("q (b k) -> q b k", k=L),
                in0=ex.rearrange("q (b k) -> q b k", k=L),
                in1=rinv[:, :, None].to_broadcast((L, BC, L)),
            )

            # per-batch transposes and attn@v, all into wide tiles
            op_all = apsum.tile([D, BC * L], f32, tag="opall")
            for b in range(BC):
                atp = apsum.tile([L, L], f32, tag="smallp", bufs=2)
                nc.tensor.transpose(atp[:], attn[:, b * L : (b + 1) * L], ident_t[:L, :L])
                ats = apool.tile([L, L], f16, tag="ats")
                nc.scalar.copy(out=ats[:], in_=atp[:])
                vtp = apsum.tile([L, D], f32, tag="smallp", bufs=2)
                nc.tensor.transpose(vtp[:], vs[:, b * L : (b + 1) * L], ident_t[:])
                vts = apool.tile([L, D], f16, tag="vts")
                nc.scalar.copy(out=vts[:], in_=vtp[:])
                nc.tensor.matmul(
                    op_all[:, b * L : (b + 1) * L],
                    lhsT=vts[:],
                    rhs=ats[:],
                    start=True,
                    stop=True,
                )
            os_all = apool.tile([D, BC * L], f16, tag="osall")
            nc.scalar.copy(out=os_all[:], in_=op_all[:])
            o2p = apsum.tile([D, BC * L], f32, tag="o2p")
            nc.tensor.matmul(o2p[:], lhsT=wo_t[:], rhs=os_all[:], start=True, stop=True)
            pooled = apool.tile([D, BC], f16, tag="pooled")
            nc.vector.reduce_max(
                out=pooled[:], in_=o2p.rearrange("d (b l) -> d b l", l=L),
                axis=mybir.AxisListType.X,
            )

            # ---- logits ----
            lgp = apsum.tile([BC, LABELS], f32, tag="smallp", bufs=2)
            nc.tensor.matmul(lgp[:], lhsT=pooled[:], rhs=wl_t[:], start=True, stop=True)
            outs = apool.tile([BC, LABELS], f32, tag="outs")
            nc.vector.tensor_add(out=outs[:], in0=lgp[:], in1=blb_t[:])
            nc.sync.dma_start(out=out_d[:], in_=outs[:])

    nc.compile()
    return nc


def _get_nc():
    if "nc" not in _CACHE:
        _CACHE["nc"] = _build_nc()
    return _CACHE["nc"]


def kernel(tokens, mask, emb, Wc, bc, Wq, Wk, Wv, Wo, Wl, bl, _trace=False):
    from concourse.bass_utils import run_bass_kernel_spmd

    tokens = np.asarray(tokens)
    mask = np.asarray(mask)
    emb16 = np.asarray(emb, dtype=np.float32).astype(np.float16)

    blb = np.tile(np.asarray(bl, np.float32)[None, :], (BC, 1))

    common = {
        "emb": emb16,
        "wc": np.asarray(Wc, np.float32),
        "bcv": np.asarray(bc, np.float32).reshape(D, 1),
        "wq": np.asarray(Wq, np.float32),
        "wk": np.asarray(Wk, np.float32),
        "wv": np.asarray(Wv, np.float32),
        "wo": np.asarray(Wo, np.float32),
        "wl": np.asarray(Wl, np.float32),
        "blb": blb,
        "ident": np.eye(128, dtype=np.float32),
    }

    in_maps = []
    for c in range(NCORES):
        tok_c = np.asarray(tokens[c * BC : (c + 1) * BC]).reshape(TREES, NPT)
        idx_lin = np.concatenate(
            [tok_c, np.zeros((TREES, 1), tok_c.dtype)], axis=1
        ).reshape(-1)
        idx_arr = np.tile(
            idx_lin.astype(np.int16).reshape(-1, 16).T, (8, 1)
        )  # [128, NIDX_TOTAL/16]
        mask_c = (
            np.asarray(mask[c * BC : (c + 1) * BC], np.int32)
            .transpose(1, 0, 2)
            .reshape(L, BC * L)
        )
        in_maps.append({**common, "idxs": idx_arr, "mask": mask_c})

    nc = _get_nc()
    res = run_bass_kernel_spmd(
        nc, in_maps, core_ids=list(range(NCORES)), trace=_trace
    )
    out = np.concatenate([r["out"] for r in res.results], axis=0)  # [B, LABELS]
    if _trace:
        return out, res
    return out

